# revision 3
# baseline (speedup 1.0000x reference)
"""Trainium2 Bass kernel for an AttentionBlock (1x1-conv QKV attention over HW).

Reference computation (per sample b):
    q = Wq @ x + bq   [QK, HW]
    k = Wk @ x + bk   [QK, HW]
    v = Wv @ x + bv   [C, HW]
    attn = softmax(q^T k, axis=j)     [HW, HW]
    out  = gamma * (v @ attn^T) + x   [C, HW]

Shapes: B=4, C=64, QK=8, H=W=64 (HW=4096), fp32.

Sharding: 8 cores = 4 samples x 2 query-row halves. Each core loads the full
sample x (for K/V) plus its half of the query rows, computes a flash-style
attention over its 2048 query rows, and writes its [64, 2048] output slice.
No collectives needed.

Per-core data flow (all matmul inputs bf16, PSUM fp32):
  x_aug  [65, 4096]  (row 64 = ones, for fused bias)
  q = (WqT|bq)^T  @ xq_aug   -> [8, 2048]   (lhsT = host-prepped [65, 8])
  k = (WkT|bk)^T  @ xf_aug   -> [8, 4096]
  vT chunks: lhsT = xf_aug[:, jc] [65,128], rhs = wv_aug [65,128]
     -> v_sb chunk [128, 128]: cols 0:64 = gamma*(Wv x + bv)^T, cols 64:128 = 1
  scores[j, i] = k_chunk^T q_tile (K=8) -> PSUM [128, 3*512]
  exp on ScalarE (the bottleneck engine): PSUM -> SBUF bf16
  AV: lhsT = v_sb chunk [128, 128], rhs = exp [128, 512], accumulate PSUM
     -> av [128, 512]: rows 0:64 = gamma*V*exp-sums, rows 64:128 = denominator
  normalize: DVE reciprocal on rows 64:128, DMA partition-shift to rows 0:64,
     DVE multiply + residual add, DMA out.
"""

import os
import sys

import numpy as np

for _p in ("/opt/trn_rl_repo", "/opt/pypackages"):
    if _p not in sys.path and os.path.isdir(_p):
        sys.path.append(_p)

import ml_dtypes  # noqa: E402

B, C, H, W = 4, 64, 64, 64
HW = H * W            # 4096
QK = C // 8           # 8
N_CORES = 8
NQ = HW // 2          # 2048 query rows per core
IT = 512              # i-tile (query) width
NIT = NQ // IT        # 4
JC = 128              # j-chunk (key) width
NJC = HW // JC        # 32
# j-chunks per exp batch: 3 chunks -> [128, 1536] fp32 = 3 PSUM banks.
# PSUM budget: scores 2x3 banks + av 2x1 banks = 8 banks.
SUPERS = [3] * 10 + [2]
assert sum(SUPERS) == NJC

_CACHE: dict = {}


def _build_bass():
    import concourse.tile as tile
    from concourse import bacc, mybir
    from concourse.bass import ts

    f32 = mybir.dt.float32
    bf16 = mybir.dt.bfloat16
    EXP = mybir.ActivationFunctionType.Exp

    nc = bacc.Bacc("TRN2", target_bir_lowering=False, debug=False)

    xf_d = nc.dram_tensor("xf", [C, HW], f32, kind="ExternalInput").ap()
    xq_d = nc.dram_tensor("xq", [C, NQ], f32, kind="ExternalInput").ap()
    wq_d = nc.dram_tensor("wq", [C + 1, QK], bf16, kind="ExternalInput").ap()
    wk_d = nc.dram_tensor("wk", [C + 1, QK], bf16, kind="ExternalInput").ap()
    wv_d = nc.dram_tensor("wv", [C + 1, JC], bf16, kind="ExternalInput").ap()
    out_d = nc.dram_tensor("out", [C, NQ], f32, kind="ExternalOutput").ap()

    with tile.TileContext(nc) as tc:
        with (
            tc.tile_pool(name="const", bufs=1) as const,
            tc.tile_pool(name="work", bufs=3) as work,
            tc.tile_pool(name="norm", bufs=2) as normp,
            tc.tile_pool(name="ps_score", bufs=2, space="PSUM") as ps_score,
            tc.tile_pool(name="ps_av", bufs=2, space="PSUM") as ps_av,
        ):
            # ---- load inputs ----
            xf32 = const.tile([C, HW], f32)
            nc.sync.dma_start(out=xf32[:], in_=xf_d[:])
            xq32 = const.tile([C, NQ], f32)
            nc.sync.dma_start(out=xq32[:], in_=xq_d[:])
            wq_t = const.tile([C + 1, QK], bf16)
            nc.sync.dma_start(out=wq_t[:], in_=wq_d[:])
            wk_t = const.tile([C + 1, QK], bf16)
            nc.sync.dma_start(out=wk_t[:], in_=wk_d[:])
            wv_t = const.tile([C + 1, JC], bf16)
            nc.sync.dma_start(out=wv_t[:], in_=wv_d[:])

            # ---- bf16 casts + ones rows (fused-bias contraction row) ----
            xf_bf = const.tile([C + 1, HW], bf16)
            nc.vector.tensor_copy(xf_bf[0:C, :], xf32[:])
            nc.gpsimd.memset(xf_bf[C : C + 1, :], 1.0)
            xq_bf = const.tile([C + 1, NQ], bf16)
            nc.vector.tensor_copy(xq_bf[0:C, :], xq32[:])
            nc.gpsimd.memset(xq_bf[C : C + 1, :], 1.0)

            # ---- projections ----
            q_sb = const.tile([QK, NQ], bf16)
            for i in range(NIT):
                qp = ps_score.tile([QK, IT], f32, tag="score")
                nc.tensor.matmul(qp[:], lhsT=wq_t[:], rhs=xq_bf[:, ts(i, IT)])
                nc.vector.tensor_copy(q_sb[:, ts(i, IT)], qp[:])

            k_sb = const.tile([QK, HW], bf16)
            for i in range(HW // IT):
                kp = ps_score.tile([QK, IT], f32, tag="score")
                nc.tensor.matmul(kp[:], lhsT=wk_t[:], rhs=xf_bf[:, ts(i, IT)])
                nc.vector.tensor_copy(k_sb[:, ts(i, IT)], kp[:])

            # vT chunks, 4 chunks of [128, 128] per PSUM bank batch
            v_sb = const.tile([JC, NJC * JC], bf16)
            for g in range(NJC // 4):
                vp = ps_score.tile([JC, 4 * JC], f32, tag="score")
                for c4 in range(4):
                    ci = 4 * g + c4
                    nc.tensor.matmul(
                        vp[:, ts(c4, JC)],
                        lhsT=xf_bf[:, ts(ci, JC)],
                        rhs=wv_t[:],
                    )
                nc.vector.tensor_copy(v_sb[:, ts(g, 4 * JC)], vp[:])

            # ---- main attention loop ----
            for t in range(NIT):
                av = ps_av.tile([JC, IT], f32, tag="av")
                ci = 0
                for ng in SUPERS:
                    sc = ps_score.tile([JC, 3 * IT], f32, tag="score")
                    for u in range(ng):
                        nc.tensor.matmul(
                            sc[:, ts(u, IT)],
                            lhsT=k_sb[:, ts(ci + u, JC)],
                            rhs=q_sb[:, ts(t, IT)],
                        )
                    ex = work.tile([JC, 3 * IT], bf16, tag="exp")
                    nc.scalar.activation(
                        ex[:, 0 : ng * IT], sc[:, 0 : ng * IT], EXP
                    )
                    for u in range(ng):
                        nc.tensor.matmul(
                            av[:],
                            lhsT=v_sb[:, ts(ci + u, JC)],
                            rhs=ex[:, ts(u, IT)],
                            start=(ci + u == 0),
                            stop=(ci + u == NJC - 1),
                        )
                    ci += ng

                # normalize + residual
                rd = normp.tile([JC, IT], f32, tag="rd")
                nc.vector.reciprocal(rd[C:JC, :], av[C:JC, :])
                db = normp.tile([C, IT], f32, tag="db")
                nc.sync.dma_start(out=db[:], in_=rd[C:JC, :])
                on = normp.tile([C, IT], f32, tag="on")
                nc.vector.tensor_mul(on[:], av[0:C, :], db[:])
                fin = work.tile([C, IT], f32, tag="fin")
                nc.vector.tensor_add(fin[:], on[:], xq32[:, ts(t, IT)])
                nc.sync.dma_start(out=out_d[:, ts(t, IT)], in_=fin[:])

    nc.compile()
    return nc


def get_nc():
    if "nc" not in _CACHE:
        _CACHE["nc"] = _build_bass()
    return _CACHE["nc"]


def make_in_maps(x, Wq, bq, Wk, bk, Wv, bv, gamma):
    x = np.asarray(x, np.float32)
    Wq = np.asarray(Wq, np.float32)
    bq = np.asarray(bq, np.float32)
    Wk = np.asarray(Wk, np.float32)
    bk = np.asarray(bk, np.float32)
    Wv = np.asarray(Wv, np.float32)
    bv = np.asarray(bv, np.float32)
    g = float(np.asarray(gamma, np.float32).reshape(-1)[0])

    bf = ml_dtypes.bfloat16
    wq_a = np.concatenate([Wq.T, bq[None, :]], axis=0).astype(bf)  # [65, 8]
    wk_a = np.concatenate([Wk.T, bk[None, :]], axis=0).astype(bf)  # [65, 8]
    # wv_a [65, 128]: cols 0:64 produce gamma*(Wv x + bv)^T, cols 64:128
    # produce the all-ones denominator rows (row 64 of x_aug is ones).
    wv_a = np.zeros((C + 1, JC), np.float32)
    wv_a[0:C, 0:C] = g * Wv.T
    wv_a[C, 0:C] = g * bv
    wv_a[C, C:JC] = 1.0
    wv_a = wv_a.astype(bf)

    in_maps = []
    for c in range(N_CORES):
        b, h = c // 2, c % 2
        xs = np.ascontiguousarray(x[b].reshape(C, HW))
        in_maps.append(
            {
                "xf": xs,
                "xq": np.ascontiguousarray(xs[:, h * NQ : (h + 1) * NQ]),
                "wq": wq_a,
                "wk": wk_a,
                "wv": wv_a,
            }
        )
    return in_maps


def assemble(results):
    out = np.empty((B, C, HW), np.float32)
    for c in range(N_CORES):
        b, h = c // 2, c % 2
        out[b][:, h * NQ : (h + 1) * NQ] = results[c]["out"]
    return out.reshape(B, C, H, W)


def kernel(x, Wq, bq, Wk, bk, Wv, bv, gamma):
    from concourse import bass_utils

    nc = get_nc()
    in_maps = make_in_maps(x, Wq, bq, Wk, bk, Wv, bv, gamma)
    res = bass_utils.run_bass_kernel_spmd(
        nc, in_maps, core_ids=list(range(N_CORES))
    )
    return assemble(res.results)
